# revision 34
# baseline (speedup 1.0000x reference)
"""GPTQ 4-bit quantized linear: out = x @ dequant(qweight, qzeros, scales, g_idx) + bias.

Full shapes: x [8192, 4096] fp16, qweight [512, 4096] int32 (8x 4-bit packed
along K), qzeros [32, 512] int32, scales [32, 4096] fp16, g_idx [4096] int32
(k // 128), bias [4096] fp16.  Output [8192, 4096] fp16.

Strategy: 2 (M) x 4 (N) grid over 8 NeuronCores.  Per core: M=4096, N=1024,
K=4096.  Host does layout only (transpose x, split qweight int32s into int16
halves, unpack the tiny qzeros to int16, expand zeros/scales rows to
partition layout).  Device dequantizes the whole weight shard once into SBUF
(64KB/partition; DVE at 2-byte rates: fused shift+mask on int16, subtract
zero with fp16 output, multiply scale) and sweeps x through it in 16
nb-waves of 128 matmuls (4 PSUM banks accumulating while the other 4 drain
via ACT copy + DVE bias-add), so x is read from HBM exactly once (46.5MB
per core vs 81MB if x were re-read per n-block).  Dequant staging and x
tiles stream on three DMA queues ordered by need-time; output stores merge
both n-halves into 2KB-line writes (GpSimd mid-kernel, SyncE for the final
wave to keep the kernel tail short).
"""

import os
import sys

import numpy as np

for _p in ("/opt/trn_rl_repo",):
    if _p not in sys.path and os.path.isdir(_p):
        sys.path.insert(0, _p)

import concourse.bass as bass
import concourse.mybir as mybir
import concourse.tile as tile
from concourse import bacc
from concourse.bass_utils import run_bass_kernel_spmd

dt = mybir.dt

P = 128          # partitions
JP = 8           # 4-bit values per int32
KA = P * JP      # k's covered by one a-chunk (1024)
NPS = 512        # psum free width
GROUP = 128      # quant group size == k-chunk size


def build_program(K, M, N):
    """One-core SPMD program: out[M,N] = xt.T @ W + bias with W dequantized
    on the fly.  xt is x-transposed [K, M]."""
    A = K // KA          # a-chunks (4)
    NB = N // NPS        # psum column blocks (2)
    WAVE_M = 4 * P       # m-rows per wave (512) -> 4 psums per nb-wave
    NW = M // WAVE_M     # waves (8)
    assert K % KA == 0 and N == 2 * NPS and M % WAVE_M == 0

    nc = bacc.Bacc("TRN2", target_bir_lowering=False)

    xt = nc.dram_tensor("xt", [K, M], dt.float16, kind="ExternalInput")
    # qweight split into int16 halves on host: row 2r+h = half h of int32 row r
    qw = nc.dram_tensor("qw", [K // 4, N], dt.int16, kind="ExternalInput")
    zp = nc.dram_tensor("zp", [K // JP, N], dt.int16, kind="ExternalInput")
    sc = nc.dram_tensor("sc", [K // JP, N], dt.float16, kind="ExternalInput")
    bs = nc.dram_tensor("bs", [P, N], dt.float16, kind="ExternalInput")
    out = nc.dram_tensor("out", [M, N], dt.float16, kind="ExternalOutput")

    # k = KA*a + 8*p + j  (p = partition, j = nibble plane)
    xt_r = xt.rearrange("(a p j) m -> a j p m", p=P, j=JP)
    # int16-half row index = 2*(128a + p) + h
    qw_r = qw.rearrange("(a p h) n -> a h p n", p=P, h=2)

    from contextlib import ExitStack

    with tile.TileContext(nc) as tc, ExitStack() as ctx:
        const = ctx.enter_context(tc.tile_pool(name="const", bufs=1))
        qpool = ctx.enter_context(tc.tile_pool(name="qpool", bufs=8))
        zpool = ctx.enter_context(tc.tile_pool(name="zpool", bufs=4))
        spool = ctx.enter_context(tc.tile_pool(name="spool", bufs=4))
        tpool = ctx.enter_context(tc.tile_pool(name="tpool", bufs=3))
        fpool = ctx.enter_context(tc.tile_pool(name="fpool", bufs=3))
        wpool = ctx.enter_context(tc.tile_pool(name="wpool", bufs=A * JP))
        xpool = ctx.enter_context(tc.tile_pool(name="xpool", bufs=40))
        opool = ctx.enter_context(tc.tile_pool(name="opool", bufs=4))
        psum = ctx.enter_context(tc.tile_pool(name="psum", bufs=8, space="PSUM"))

        # PE warmup: dummy matmuls with a single cheap memset dependency,
        # issued during the framework preamble so the HAM clock-gate opens
        # before the first real matmul and the PE has work while the first
        # dequant inputs stream in.
        warm_src = const.tile([P, NPS], dt.float16, tag="warm")
        nc.gpsimd.memset(warm_src[:], 0.0)
        warm_ps = psum.tile([P, NPS], dt.float32, tag="ps")
        NWARM = 10
        for wi in range(NWARM):
            nc.tensor.matmul(
                warm_ps[:], warm_src[:, :P], warm_src[:],
                start=(wi == 0), stop=(wi == NWARM - 1),
            )

        bias_t = const.tile([P, N], dt.float16, tag="bias")

        # ---- dequant-input staging (full-N tiles, 2KB DMA lines) --------
        # a0's qweight rides the fast SyncE queue (critical path to the
        # first matmul); a0's zeros/scales lead GpSimd's queue in parallel.
        # bias is only needed by the ACT psum-seeds from ~wave-0's drains
        # on, so it rides mid-queue.
        qh, zh, sh = {}, {}, {}

        def load_q(a, h, eng):
            q_t = qpool.tile([P, N], dt.int16, tag="qh")
            eng.dma_start(q_t[:], qw_r[a, h, :, :])
            qh[(a, h)] = q_t

        def load_zs(a, eng):
            z_t = zpool.tile([P, N], dt.int16, tag="zh")
            eng.dma_start(z_t[:], zp[a * P:(a + 1) * P, :])
            s_t = spool.tile([P, N], dt.float16, tag="sh")
            eng.dma_start(s_t[:], sc[a * P:(a + 1) * P, :])
            zh[a] = z_t
            sh[a] = s_t

        # need-time ordered across three queues: SyncE carries a0's qweight
        # then the even wave-0 x tiles; GpSimd carries the early qweights
        # interleaved with odd wave-0 x tiles (emitted below); ACT carries
        # zeros/scales/bias and the late qweights, finishing before its
        # drain work starts (~38us).
        # a0's tiles load in column-halves so the first dequant ops (which
        # only read cols 0:512) start as early as possible.
        qh[(0, 0)] = qpool.tile([P, N], dt.int16, tag="qh", name="qh00")
        nc.sync.dma_start(qh[(0, 0)][:, 0:NPS], qw_r[0, 0, :, 0:NPS])
        nc.sync.dma_start(qh[(0, 0)][:, NPS:N], qw_r[0, 0, :, NPS:N])
        zh[0] = zpool.tile([P, N], dt.int16, tag="zh", name="zh0")
        sh[0] = spool.tile([P, N], dt.float16, tag="sh", name="sh0")
        nc.scalar.dma_start(zh[0][:, 0:NPS], zp[0:P, 0:NPS])
        nc.scalar.dma_start(sh[0][:, 0:NPS], sc[0:P, 0:NPS])
        nc.scalar.dma_start(zh[0][:, NPS:N], zp[0:P, NPS:N])
        nc.scalar.dma_start(sh[0][:, NPS:N], sc[0:P, NPS:N])
        nc.scalar.dma_start(bias_t[:], bs[:])
        load_q(0, 1, nc.gpsimd)
        load_zs(1, nc.scalar)
        load_q(1, 0, nc.gpsimd)
        load_q(1, 1, nc.gpsimd)
        load_q(2, 0, nc.scalar)
        load_zs(2, nc.scalar)
        load_q(2, 1, nc.scalar)
        load_q(3, 0, nc.scalar)
        load_zs(3, nc.scalar)
        load_q(3, 1, nc.scalar)

        # ---- dequant: all of W -> SBUF-resident fp16 -------------------
        # nb0 halves first so wave-0 (which consumes nb0) is never starved.
        W = {}
        for a in range(A):
            for j in range(JP):
                W[(a, j)] = wpool.tile([P, N], dt.float16, tag="w", name=f"w_{a}_{j}")

        def dequant_half(a, j, nb):
            # j = 4h + j2: nibble j2 of int16 half h
            h, j2 = divmod(j, 4)
            ncol = slice(nb * NPS, (nb + 1) * NPS)
            ti = tpool.tile([P, NPS], dt.int16, tag="ti")
            nc.vector.tensor_scalar(
                ti[:], qh[(a, h)][:, ncol], 4 * j2, 15,
                op0=mybir.AluOpType.logical_shift_right,
                op1=mybir.AluOpType.bitwise_and,
            )
            tf = fpool.tile([P, NPS], dt.float16, tag="tf")
            nc.vector.tensor_tensor(
                tf[:], ti[:], zh[a][:, ncol], op=mybir.AluOpType.subtract
            )
            nc.vector.tensor_tensor(
                W[(a, j)][:, ncol], tf[:], sh[a][:, ncol],
                op=mybir.AluOpType.mult,
            )

        # wave-0 x loads are interleaved with the nb0 dequant emission so
        # each queue's issue order matches PE consumption order (evens on
        # SyncE, odds on GpSimd between the early qweight loads).
        wave0_xts = {}
        p_i = 0
        for a in range(A):
            for j in range(JP):
                dequant_half(a, j, 0)
                x_t = xpool.tile([P, WAVE_M], dt.float16, tag="x_t")
                (nc.sync if p_i % 2 == 0 else nc.gpsimd).dma_start(
                    x_t[:], xt_r[a, j, :, 0:WAVE_M]
                )
                wave0_xts[(a, j)] = x_t
                p_i += 1
        for a in range(A):
            for j in range(JP):
                dequant_half(a, j, 1)

        planes = [(a, j) for a in range(A) for j in range(JP)]
        NPLANE = len(planes)  # 32

        # ---- waves ------------------------------------------------------
        # Each nb-wave t = (w, nb) runs 128 matmuls (a proper start=True
        # accumulation group per bank) on 4 PSUM banks while the other 4
        # banks drain: ACT copy -> oc frees the bank, DVE adds bias into
        # the merged ob tile, and the store rides GpSimd mid-kernel (SyncE
        # for the final wave, whose GpSimd queue-flush would otherwise sit
        # on the kernel tail).
        xts = dict(wave0_xts)
        obs = {}
        nbwaves = [(w, nb) for w in range(NW) for nb in range(NB)]
        NT = len(nbwaves)  # 16

        for t, (w, nb) in enumerate(nbwaves):
            mbase = w * WAVE_M
            ncol = slice(nb * NPS, (nb + 1) * NPS)
            pss = [psum.tile([P, NPS], dt.float32, tag="ps", name=f"ps_{t}_{i}")
                   for i in range(4)]
            last_nbwave = (t == NT - 1)

            def mm(msub, idx):
                a, j = planes[idx]
                nc.tensor.matmul(
                    pss[msub][:],
                    xts[(a, j)][:, msub * P:(msub + 1) * P],
                    W[(a, j)][:, ncol],
                    start=(idx == 0),
                    stop=(idx == NPLANE - 1),
                )

            def drain(msub, store_eng=None):
                # ob[msub] collects both nb halves -> one 2KB-line store
                if nb == 0:
                    ob = opool.tile([P, N], dt.float16, tag="ob")
                    obs[msub] = ob
                ob = obs[msub]
                oc = opool.tile([P, NPS], dt.float16, tag="oc")
                nc.scalar.copy(oc[:], pss[msub][:])
                nc.vector.tensor_tensor(
                    ob[:, ncol], oc[:], bias_t[:, ncol],
                    op=mybir.AluOpType.add,
                )
                if nb == NB - 1:
                    eng = store_eng if store_eng is not None else nc.gpsimd
                    eng.dma_start(
                        out[mbase + msub * P: mbase + (msub + 1) * P, :],
                        ob[:],
                    )

            if last_nbwave:
                # msub-major: psums finish one at a time so the drains +
                # stores overlap the remaining matmuls.  Drain directly on
                # DVE (psum + bias in one op, idle engine) and store on
                # SyncE to keep the kernel tail short.
                for msub in range(4):
                    for idx in range(NPLANE):
                        mm(msub, idx)
                    ob = obs[msub]
                    nc.vector.tensor_tensor(
                        ob[:, ncol], pss[msub][:], bias_t[:, ncol],
                        op=mybir.AluOpType.add,
                    )
                    nc.sync.dma_start(
                        out[mbase + msub * P: mbase + (msub + 1) * P, :],
                        ob[:],
                    )
            else:
                # plane-major: each fresh x/W pair feeds 4 matmuls; x
                # tiles free progressively for the next wave's prefetch.
                for idx in range(NPLANE):
                    for msub in range(4):
                        mm(msub, idx)
                    # next-wave x prefetch in consumption order, during the
                    # nb1 phase only (a plane's tile is dead for this wave
                    # right after its nb1 matmuls).  Alternate SyncE/GpSimd
                    # queues to sustain the 32-tile burst.
                    if w + 1 < NW and nb == NB - 1:
                        a, j = planes[idx]
                        x_t = xpool.tile([P, WAVE_M], dt.float16, tag="x_t")
                        (nc.sync if idx % 2 == 0 else nc.gpsimd).dma_start(
                            x_t[:],
                            xt_r[a, j, :, (w + 1) * WAVE_M:(w + 2) * WAVE_M],
                        )
                        xts[(a, j)] = x_t
                for msub in range(4):
                    drain(msub)
    nc.finalize()
    return nc


def host_prep(x, qweight, qzeros, scales, g_idx, bias, m_split, n_split):
    """Slice + lay out the full inputs into 8 per-core input maps."""
    M_full, K = x.shape
    G, N_full = scales.shape
    M = M_full // m_split
    N = N_full // n_split

    shifts = (np.arange(JP, dtype=np.int32) * 4)
    z = ((qzeros[:, :, None] >> shifts[None, None, :]) & 15).reshape(G, N_full)
    z = (z.astype(np.int32) + 1).astype(np.int16)

    # group id per k-chunk of 128 (reference always uses g_idx = k // 128)
    cg = np.asarray(g_idx[::GROUP])
    assert np.array_equal(np.repeat(cg, GROUP), np.asarray(g_idx)), \
        "g_idx must be uniform within 128-wide k chunks"
    z_c = z[cg]                       # [K/128, N_full] int16
    s_c = np.asarray(scales)[cg]      # [K/128, N_full] fp16
    zp_full = np.repeat(z_c, 16, axis=0)   # [K/8, N_full], row 16c+t -> chunk c
    sc_full = np.repeat(s_c, 16, axis=0)

    xT = np.ascontiguousarray(np.asarray(x).T)  # [K, M_full]
    qweight = np.asarray(qweight)
    # split each packed int32 into (low16, high16) rows: row 2r+h = half h
    qw16 = (
        qweight.view(np.int16)
        .reshape(qweight.shape[0], N_full, 2)
        .transpose(0, 2, 1)
        .reshape(qweight.shape[0] * 2, N_full)
    )
    bias = np.asarray(bias)

    in_maps = []
    for mi in range(m_split):
        for ni in range(n_split):
            nsl = slice(ni * N, (ni + 1) * N)
            in_maps.append({
                "xt": np.ascontiguousarray(xT[:, mi * M:(mi + 1) * M]),
                "qw": np.ascontiguousarray(qw16[:, nsl]),
                "zp": np.ascontiguousarray(zp_full[:, nsl]),
                "sc": np.ascontiguousarray(sc_full[:, nsl]),
                "bs": np.ascontiguousarray(
                    np.broadcast_to(bias[nsl], (P, N))
                ),
            })
    return in_maps, M, N


_PROGRAM_CACHE = {}


def _get_program(K, M, N):
    key = (K, M, N)
    if key not in _PROGRAM_CACHE:
        _PROGRAM_CACHE[key] = build_program(K, M, N)
    return _PROGRAM_CACHE[key]


def kernel(x, qweight, qzeros, scales, g_idx, bias, trace=False, trace_kwargs=None):
    m_split, n_split = 2, 4
    x = np.asarray(x)
    qweight = np.asarray(qweight)
    qzeros = np.asarray(qzeros)
    scales = np.asarray(scales)
    g_idx = np.asarray(g_idx)
    bias = np.asarray(bias)
    M_full, K = x.shape
    N_full = scales.shape[1]
    in_maps, M, N = host_prep(x, qweight, qzeros, scales, g_idx, bias,
                              m_split, n_split)
    nc = _get_program(K, M, N)
    kw = {}
    if trace:
        kw = dict(trace=True, **(trace_kwargs or {}))
    rb = run_bass_kernel_spmd(nc, in_maps, list(range(m_split * n_split)), **kw)
    out = np.empty((M_full, N_full), dtype=np.float16)
    ci = 0
    for mi in range(m_split):
        for ni in range(n_split):
            out[mi * M:(mi + 1) * M, ni * N:(ni + 1) * N] = rb.results[ci]["out"]
            ci += 1
    kernel.last_results = rb
    return out


# revision 36
# speedup vs baseline: 1.0047x; 1.0047x over previous
"""GPTQ 4-bit quantized linear: out = x @ dequant(qweight, qzeros, scales, g_idx) + bias.

Full shapes: x [8192, 4096] fp16, qweight [512, 4096] int32 (8x 4-bit packed
along K), qzeros [32, 512] int32, scales [32, 4096] fp16, g_idx [4096] int32
(k // 128), bias [4096] fp16.  Output [8192, 4096] fp16.

Strategy: 2 (M) x 4 (N) grid over 8 NeuronCores.  Per core: M=4096, N=1024,
K=4096.  Host does layout only (transpose x, split qweight int32s into int16
halves, unpack the tiny qzeros to int16, expand zeros/scales rows to
partition layout).  Device dequantizes the whole weight shard once into SBUF
(64KB/partition; DVE at 2-byte rates: fused shift+mask on int16, subtract
zero with fp16 output, multiply scale) and sweeps x through it in 16
nb-waves of 128 matmuls (4 PSUM banks accumulating while the other 4 drain
via ACT copy + DVE bias-add), so x is read from HBM exactly once (46.5MB
per core vs 81MB if x were re-read per n-block).  Dequant staging and x
tiles stream on three DMA queues ordered by need-time; output stores merge
both n-halves into 2KB-line writes (GpSimd mid-kernel, SyncE for the final
wave to keep the kernel tail short).
"""

import os
import sys

import numpy as np

for _p in ("/opt/trn_rl_repo",):
    if _p not in sys.path and os.path.isdir(_p):
        sys.path.insert(0, _p)

import concourse.bass as bass
import concourse.mybir as mybir
import concourse.tile as tile
from concourse import bacc
from concourse.bass_utils import run_bass_kernel_spmd

dt = mybir.dt

P = 128          # partitions
JP = 8           # 4-bit values per int32
KA = P * JP      # k's covered by one a-chunk (1024)
NPS = 512        # psum free width
GROUP = 128      # quant group size == k-chunk size


def build_program(K, M, N):
    """One-core SPMD program: out[M,N] = xt.T @ W + bias with W dequantized
    on the fly.  xt is x-transposed [K, M]."""
    A = K // KA          # a-chunks (4)
    NB = N // NPS        # psum column blocks (2)
    WAVE_M = 4 * P       # m-rows per wave (512) -> 4 psums per nb-wave
    NW = M // WAVE_M     # waves (8)
    assert K % KA == 0 and N == 2 * NPS and M % WAVE_M == 0

    nc = bacc.Bacc("TRN2", target_bir_lowering=False)

    xt = nc.dram_tensor("xt", [K, M], dt.float16, kind="ExternalInput")
    # qweight split into int16 halves on host: row 2r+h = half h of int32 row r
    qw = nc.dram_tensor("qw", [K // 4, N], dt.int16, kind="ExternalInput")
    zp = nc.dram_tensor("zp", [K // JP, N], dt.int16, kind="ExternalInput")
    sc = nc.dram_tensor("sc", [K // JP, N], dt.float16, kind="ExternalInput")
    bs = nc.dram_tensor("bs", [P, N], dt.float16, kind="ExternalInput")
    out = nc.dram_tensor("out", [M, N], dt.float16, kind="ExternalOutput")

    # k = KA*a + 8*p + j  (p = partition, j = nibble plane)
    xt_r = xt.rearrange("(a p j) m -> a j p m", p=P, j=JP)
    # int16-half row index = 2*(128a + p) + h
    qw_r = qw.rearrange("(a p h) n -> a h p n", p=P, h=2)

    from contextlib import ExitStack

    with tile.TileContext(nc) as tc, ExitStack() as ctx:
        const = ctx.enter_context(tc.tile_pool(name="const", bufs=1))
        qpool = ctx.enter_context(tc.tile_pool(name="qpool", bufs=8))
        zpool = ctx.enter_context(tc.tile_pool(name="zpool", bufs=4))
        spool = ctx.enter_context(tc.tile_pool(name="spool", bufs=4))
        tpool = ctx.enter_context(tc.tile_pool(name="tpool", bufs=3))
        fpool = ctx.enter_context(tc.tile_pool(name="fpool", bufs=3))
        wpool = ctx.enter_context(tc.tile_pool(name="wpool", bufs=A * JP))
        xpool = ctx.enter_context(tc.tile_pool(name="xpool", bufs=40))
        opool = ctx.enter_context(tc.tile_pool(name="opool", bufs=4))
        psum = ctx.enter_context(tc.tile_pool(name="psum", bufs=8, space="PSUM"))

        # PE warmup: dummy matmuls with a single cheap memset dependency,
        # issued during the framework preamble so the HAM clock-gate opens
        # before the first real matmul and the PE has work while the first
        # dequant inputs stream in.
        warm_src = const.tile([P, NPS], dt.float16, tag="warm")
        nc.gpsimd.memset(warm_src[:], 0.0)
        warm_ps = psum.tile([P, NPS], dt.float32, tag="ps")
        NWARM = 18
        for wi in range(NWARM):
            nc.tensor.matmul(
                warm_ps[:], warm_src[:, :P], warm_src[:],
                start=(wi == 0), stop=(wi == NWARM - 1),
            )

        bias_t = const.tile([P, N], dt.float16, tag="bias")

        # ---- dequant-input staging (full-N tiles, 2KB DMA lines) --------
        # a0's qweight rides the fast SyncE queue (critical path to the
        # first matmul); a0's zeros/scales lead GpSimd's queue in parallel.
        # bias is only needed by the ACT psum-seeds from ~wave-0's drains
        # on, so it rides mid-queue.
        qh, zh, sh = {}, {}, {}

        def load_q(a, h, eng):
            q_t = qpool.tile([P, N], dt.int16, tag="qh")
            eng.dma_start(q_t[:], qw_r[a, h, :, :])
            qh[(a, h)] = q_t

        def load_zs(a, eng):
            z_t = zpool.tile([P, N], dt.int16, tag="zh")
            eng.dma_start(z_t[:], zp[a * P:(a + 1) * P, :])
            s_t = spool.tile([P, N], dt.float16, tag="sh")
            eng.dma_start(s_t[:], sc[a * P:(a + 1) * P, :])
            zh[a] = z_t
            sh[a] = s_t

        # need-time ordered across three queues: SyncE carries a0's qweight
        # then the even wave-0 x tiles; GpSimd carries the early qweights
        # interleaved with odd wave-0 x tiles (emitted below); ACT carries
        # zeros/scales/bias and the late qweights, finishing before its
        # drain work starts (~38us).
        load_q(0, 0, nc.sync)
        load_zs(0, nc.scalar)
        nc.scalar.dma_start(bias_t[:], bs[:])
        load_q(0, 1, nc.gpsimd)
        load_zs(1, nc.scalar)
        load_q(1, 0, nc.gpsimd)
        load_q(1, 1, nc.gpsimd)
        load_q(2, 0, nc.scalar)
        load_zs(2, nc.scalar)
        load_q(2, 1, nc.scalar)
        load_q(3, 0, nc.scalar)
        load_zs(3, nc.scalar)
        load_q(3, 1, nc.scalar)

        # ---- dequant: all of W -> SBUF-resident fp16 -------------------
        # nb0 halves first so wave-0 (which consumes nb0) is never starved.
        W = {}
        for a in range(A):
            for j in range(JP):
                W[(a, j)] = wpool.tile([P, N], dt.float16, tag="w", name=f"w_{a}_{j}")

        def dequant_half(a, j, nb):
            # j = 4h + j2: nibble j2 of int16 half h
            h, j2 = divmod(j, 4)
            ncol = slice(nb * NPS, (nb + 1) * NPS)
            ti = tpool.tile([P, NPS], dt.int16, tag="ti")
            nc.vector.tensor_scalar(
                ti[:], qh[(a, h)][:, ncol], 4 * j2, 15,
                op0=mybir.AluOpType.logical_shift_right,
                op1=mybir.AluOpType.bitwise_and,
            )
            tf = fpool.tile([P, NPS], dt.float16, tag="tf")
            nc.vector.tensor_tensor(
                tf[:], ti[:], zh[a][:, ncol], op=mybir.AluOpType.subtract
            )
            nc.vector.tensor_tensor(
                W[(a, j)][:, ncol], tf[:], sh[a][:, ncol],
                op=mybir.AluOpType.mult,
            )

        # wave-0 x loads are interleaved with the nb0 dequant emission so
        # each queue's issue order matches PE consumption order (evens on
        # SyncE, odds on GpSimd between the early qweight loads).
        wave0_xts = {}
        p_i = 0
        for a in range(A):
            for j in range(JP):
                dequant_half(a, j, 0)
                x_t = xpool.tile([P, WAVE_M], dt.float16, tag="x_t")
                (nc.sync if p_i % 2 == 0 else nc.gpsimd).dma_start(
                    x_t[:], xt_r[a, j, :, 0:WAVE_M]
                )
                wave0_xts[(a, j)] = x_t
                p_i += 1
        for a in range(A):
            for j in range(JP):
                dequant_half(a, j, 1)

        planes = [(a, j) for a in range(A) for j in range(JP)]
        NPLANE = len(planes)  # 32

        # ---- waves ------------------------------------------------------
        # Each nb-wave t = (w, nb) runs 128 matmuls (a proper start=True
        # accumulation group per bank) on 4 PSUM banks while the other 4
        # banks drain: ACT copy -> oc frees the bank, DVE adds bias into
        # the merged ob tile, and the store rides GpSimd mid-kernel (SyncE
        # for the final wave, whose GpSimd queue-flush would otherwise sit
        # on the kernel tail).
        xts = dict(wave0_xts)
        obs = {}
        nbwaves = [(w, nb) for w in range(NW) for nb in range(NB)]
        NT = len(nbwaves)  # 16

        for t, (w, nb) in enumerate(nbwaves):
            mbase = w * WAVE_M
            ncol = slice(nb * NPS, (nb + 1) * NPS)
            pss = [psum.tile([P, NPS], dt.float32, tag="ps", name=f"ps_{t}_{i}")
                   for i in range(4)]
            last_nbwave = (t == NT - 1)

            def mm(msub, idx):
                a, j = planes[idx]
                nc.tensor.matmul(
                    pss[msub][:],
                    xts[(a, j)][:, msub * P:(msub + 1) * P],
                    W[(a, j)][:, ncol],
                    start=(idx == 0),
                    stop=(idx == NPLANE - 1),
                )

            def drain(msub, store_eng=None):
                # ob[msub] collects both nb halves -> one 2KB-line store
                if nb == 0:
                    ob = opool.tile([P, N], dt.float16, tag="ob")
                    obs[msub] = ob
                ob = obs[msub]
                oc = opool.tile([P, NPS], dt.float16, tag="oc")
                nc.scalar.copy(oc[:], pss[msub][:])
                nc.vector.tensor_tensor(
                    ob[:, ncol], oc[:], bias_t[:, ncol],
                    op=mybir.AluOpType.add,
                )
                if nb == NB - 1:
                    eng = store_eng if store_eng is not None else nc.gpsimd
                    eng.dma_start(
                        out[mbase + msub * P: mbase + (msub + 1) * P, :],
                        ob[:],
                    )
                elif w == NW - 1:
                    # final wave: ship the nb0 half early on GpSimd so the
                    # last nb-wave only stores 128KB halves on SyncE,
                    # shortening the kernel-tail critical path.
                    nc.gpsimd.dma_start(
                        out[mbase + msub * P: mbase + (msub + 1) * P, 0:NPS],
                        ob[:, 0:NPS],
                    )

            if last_nbwave:
                # msub-major: psums finish one at a time so the drains +
                # stores overlap the remaining matmuls.  Drain directly on
                # DVE (psum + bias in one op, idle engine) and store on
                # SyncE to keep the kernel tail short.
                for msub in range(4):
                    for idx in range(NPLANE):
                        mm(msub, idx)
                    ob = obs[msub]
                    nc.vector.tensor_tensor(
                        ob[:, ncol], pss[msub][:], bias_t[:, ncol],
                        op=mybir.AluOpType.add,
                    )
                    nc.sync.dma_start(
                        out[mbase + msub * P: mbase + (msub + 1) * P, NPS:N],
                        ob[:, NPS:N],
                    )
            else:
                # plane-major: each fresh x/W pair feeds 4 matmuls; x
                # tiles free progressively for the next wave's prefetch.
                for idx in range(NPLANE):
                    for msub in range(4):
                        mm(msub, idx)
                    # next-wave x prefetch in consumption order, during the
                    # nb1 phase only (a plane's tile is dead for this wave
                    # right after its nb1 matmuls).  Alternate SyncE/GpSimd
                    # queues to sustain the 32-tile burst.
                    if w + 1 < NW and nb == NB - 1:
                        a, j = planes[idx]
                        x_t = xpool.tile([P, WAVE_M], dt.float16, tag="x_t")
                        (nc.sync if idx % 2 == 0 else nc.gpsimd).dma_start(
                            x_t[:],
                            xt_r[a, j, :, (w + 1) * WAVE_M:(w + 2) * WAVE_M],
                        )
                        xts[(a, j)] = x_t
                for msub in range(4):
                    drain(msub)
    nc.finalize()
    return nc


def host_prep(x, qweight, qzeros, scales, g_idx, bias, m_split, n_split):
    """Slice + lay out the full inputs into 8 per-core input maps."""
    M_full, K = x.shape
    G, N_full = scales.shape
    M = M_full // m_split
    N = N_full // n_split

    shifts = (np.arange(JP, dtype=np.int32) * 4)
    z = ((qzeros[:, :, None] >> shifts[None, None, :]) & 15).reshape(G, N_full)
    z = (z.astype(np.int32) + 1).astype(np.int16)

    # group id per k-chunk of 128 (reference always uses g_idx = k // 128)
    cg = np.asarray(g_idx[::GROUP])
    assert np.array_equal(np.repeat(cg, GROUP), np.asarray(g_idx)), \
        "g_idx must be uniform within 128-wide k chunks"
    z_c = z[cg]                       # [K/128, N_full] int16
    s_c = np.asarray(scales)[cg]      # [K/128, N_full] fp16
    zp_full = np.repeat(z_c, 16, axis=0)   # [K/8, N_full], row 16c+t -> chunk c
    sc_full = np.repeat(s_c, 16, axis=0)

    xT = np.ascontiguousarray(np.asarray(x).T)  # [K, M_full]
    qweight = np.asarray(qweight)
    # split each packed int32 into (low16, high16) rows: row 2r+h = half h
    qw16 = (
        qweight.view(np.int16)
        .reshape(qweight.shape[0], N_full, 2)
        .transpose(0, 2, 1)
        .reshape(qweight.shape[0] * 2, N_full)
    )
    bias = np.asarray(bias)

    in_maps = []
    for mi in range(m_split):
        for ni in range(n_split):
            nsl = slice(ni * N, (ni + 1) * N)
            in_maps.append({
                "xt": np.ascontiguousarray(xT[:, mi * M:(mi + 1) * M]),
                "qw": np.ascontiguousarray(qw16[:, nsl]),
                "zp": np.ascontiguousarray(zp_full[:, nsl]),
                "sc": np.ascontiguousarray(sc_full[:, nsl]),
                "bs": np.ascontiguousarray(
                    np.broadcast_to(bias[nsl], (P, N))
                ),
            })
    return in_maps, M, N


_PROGRAM_CACHE = {}


def _get_program(K, M, N):
    key = (K, M, N)
    if key not in _PROGRAM_CACHE:
        _PROGRAM_CACHE[key] = build_program(K, M, N)
    return _PROGRAM_CACHE[key]


def kernel(x, qweight, qzeros, scales, g_idx, bias, trace=False, trace_kwargs=None):
    m_split, n_split = 2, 4
    x = np.asarray(x)
    qweight = np.asarray(qweight)
    qzeros = np.asarray(qzeros)
    scales = np.asarray(scales)
    g_idx = np.asarray(g_idx)
    bias = np.asarray(bias)
    M_full, K = x.shape
    N_full = scales.shape[1]
    in_maps, M, N = host_prep(x, qweight, qzeros, scales, g_idx, bias,
                              m_split, n_split)
    nc = _get_program(K, M, N)
    kw = {}
    if trace:
        kw = dict(trace=True, **(trace_kwargs or {}))
    rb = run_bass_kernel_spmd(nc, in_maps, list(range(m_split * n_split)), **kw)
    out = np.empty((M_full, N_full), dtype=np.float16)
    ci = 0
    for mi in range(m_split):
        for ni in range(n_split):
            out[mi * M:(mi + 1) * M, ni * N:(ni + 1) * N] = rb.results[ci]["out"]
            ci += 1
    kernel.last_results = rb
    return out
